# revision 4
# baseline (speedup 1.0000x reference)
"""Dense attention kernel for Trainium2, 8 NeuronCores (SPMD).

Problem: q,k,v [8192, 1024] fp32; out = softmax(q @ k.T / sqrt(1024)) @ v.

Strategy (sequence-parallel over q, per the sharding hint):
  - Core c owns q rows [c*1024, (c+1)*1024); k and v are replicated.
  - Host pre-transposes: each core receives qT [D, M]=[1024, 1024] (its q
    shard transposed) and kT [D, N]=[1024, 8192] (k transposed), so the
    contraction dim D is the SBUF partition dim for both matmul operands
    and no on-chip transposes are needed anywhere.
  - Scores are computed TRANSPOSED: sT[n, m] = sum_d kT[d, n] * qT[d, m]
    (lhsT = kT chunk, rhs = qT chunk). The softmax numerator
    pT = exp(sT / 32) then already has the kv dim n on partitions, which is
    exactly the lhsT layout the second matmul needs: o[m, j] += pT.T @ v.
  - No running max: scores/32 ~ N(0,1), max over 8192 ~ 4.3, so exp() is
    bounded by ~e^5 — no overflow risk in fp32, and softmax is shift
    invariant so the result matches the reference.
  - The softmax denominator l[m] = sum_n pT[n, m] falls out of a 1-column
    matmul against a ones vector, accumulated in PSUM alongside o.
  - In the last kv block, finalization (l add, reciprocal, scale, store) is
    fused per m-tile so the tail pipelines instead of serializing.

kv is streamed once per core in blocks of NB columns; o/l accumulate in
SBUF fp32 across blocks.

The executor mirrors concourse.bass2jax.run_bass_via_pjrt but caches the
jitted computation (run_bass_via_pjrt re-traces per call). `reps` unrolls
the whole attention pass inside the module for steady-state timing.
"""

import numpy as np
import ml_dtypes

# ---- problem geometry (hardcoded per contract) ----
N = 8192
D = 1024
NCORES = 8
M = N // NCORES  # 1024 q rows per core
P = 128
DC = D // P  # 8 contraction chunks

SCALE = 1.0 / np.sqrt(np.float32(D))

# Structure knobs (A/B tested on HW via bench_kernel.py):
#   nb       — kv block columns streamed per phase
#   mhw      — rhs (moving operand) width per scores matmul
#   ojw      — rhs width per output matmul
#   sps/ops  — PSUM pool buffer counts for scores / output phases
#   kt_split — per-dc DMA split of every kT block (not just b=0)
DEFAULT_CFG = dict(
    mm_dtype="bf16",
    nb=512,
    mhw=512,
    ojw=512,
    sps_bufs=2,
    # 5 output-phase PSUM banks (2+5+1=8 total): measured ~570 vs ~600
    # us/pass at 4 across three interleaved loop-slope A/B sessions; deeper
    # rotation lets one more m-tile's matmuls start before a prior tile's
    # psum drain completes. 3 is catastrophically slower (breaks the
    # same-weight matmul trio placement).
    ops_bufs=5,
    # Triple-buffered pT: lets block b+1's exp outputs land while block
    # b-1's output-phase matmuls still read their pT (measured ~584 vs
    # ~605 us/pass in two independent loop-slope A/B sessions).
    pt_bufs=3,
    kt_split=False,
    oj_major=False,
    # 4-deep finalization-tile rotation: lets more per-m-tile l/rcp/scale
    # chains overlap the last block's out-phase matmuls. Nominally best in
    # two independent 10/22-round interleaved A/B sessions (q25 555.8/559.1
    # vs control 557.2/574.8 us); never worse than control beyond noise.
    fin_bufs=4,
)

CFG = dict(DEFAULT_CFG)

_cache = {}


def _build(cfg, reps=1):
    import concourse.bass as bass
    import concourse.tile as tile
    import concourse.mybir as mybir
    from concourse import bacc

    f32 = mybir.dt.float32
    if cfg["mm_dtype"] == "bf16":
        mdt = mybir.dt.bfloat16
        mmcast = lambda ap: ap
    else:
        mdt = mybir.dt.float32
        mmcast = lambda ap: ap.bitcast(mybir.dt.float32r)

    NB = cfg["nb"]
    NBLK = N // NB
    NCX = NB // P          # kv partition-chunks per block
    MTS = M // P           # m-tiles per core
    MHW = cfg["mhw"]       # scores moving-operand width
    NMH = M // MHW
    OJW = cfg["ojw"]       # out-phase moving-operand width
    NOJ = D // OJW

    # disable_frame_to_traceback keeps caller tracebacks out of the BIR so
    # the build (and the NEFF-cache key) is identical from any call site.
    nc = bacc.Bacc("TRN2", target_bir_lowering=False, debug=False,
                   num_devices=NCORES, disable_frame_to_traceback=True)
    qT_d = nc.declare_dram_parameter("qT", [D, M], mdt, isOutput=False)
    kT_d = nc.declare_dram_parameter("kT", [D, N], mdt, isOutput=False)
    v_d = nc.declare_dram_parameter("v", [N, D], mdt, isOutput=False)
    o_d = nc.declare_dram_parameter("o", [M, D], f32, isOutput=True)

    qT_r = qT_d.rearrange("(dc p) m -> p dc m", p=P)
    kT_r = kT_d.rearrange("(dc p) n -> p dc n", p=P)
    v_r = v_d.rearrange("(nb p) j -> p nb j", p=P)
    o_r = o_d.rearrange("(mt p) j -> p mt j", p=P)

    Exp = mybir.ActivationFunctionType.Exp

    wide = cfg["mm_dtype"] != "bf16"
    qabufs = cfg.get("qa_bufs") or (1 if (reps == 1 or wide) else 2)
    finbufs = cfg.get("fin_bufs", 2)
    kvbufs = cfg.get("kv_bufs") or (2 if (wide or NB >= 1024) else 3)

    with tile.TileContext(nc) as tc:
        with (
            tc.tile_pool(name="const", bufs=1) as cpool,
            tc.tile_pool(name="qT", bufs=qabufs) as qpool,
            tc.tile_pool(name="acc", bufs=qabufs) as apool,
            tc.tile_pool(name="kT", bufs=kvbufs) as kpool,
            tc.tile_pool(name="v", bufs=kvbufs) as vpool,
            tc.tile_pool(name="pT", bufs=cfg.get("pt_bufs", 2)) as ppool,
            tc.tile_pool(name="fin", bufs=finbufs) as fpool,
            tc.tile_pool(name="sps", bufs=cfg["sps_bufs"], space="PSUM") as spsum,
            tc.tile_pool(name="ops", bufs=cfg["ops_bufs"], space="PSUM") as opsum,
            tc.tile_pool(name="lps", bufs=1, space="PSUM") as lpsum,
        ):
            ones = cpool.tile([P, 1], mdt)
            nc.vector.memset(ones[:], 1.0)

            def emit_pass(first_pass):
                # Per-dc DMA split so the first matmuls start as soon as the
                # first contraction chunks land, not after the whole 2 MB.
                qT_sb = qpool.tile([P, DC, M], mdt, name="qT_sb")
                kT_b0 = None
                if first_pass:
                    # First pass: interleave qT and kT(b=0) chunk DMAs on the
                    # (FIFO) HWDGE ring so the first score chain's dc=0
                    # operands land ~1 us in, not after the whole 2 MB of qT.
                    kT_b0 = kpool.tile([P, DC, NB], mdt, name="kT_blk")
                    for dc in range(DC):
                        nc.sync.dma_start(qT_sb[:, dc, :], qT_r[:, dc, :])
                        nc.sync.dma_start(kT_b0[:, dc, :],
                                          kT_r[:, dc, 0:NB])
                else:
                    for dc in range(DC):
                        nc.sync.dma_start(qT_sb[:, dc, :], qT_r[:, dc, :])

                o_acc = apool.tile([P, MTS, D], f32, name="o_acc")
                l_acc = apool.tile([P, MTS], f32)

                for b in range(NBLK):
                    last = b == NBLK - 1
                    if b == 0 and kT_b0 is not None:
                        kT_blk = kT_b0
                    else:
                        kT_blk = kpool.tile([P, DC, NB], mdt)
                        if cfg["kt_split"]:
                            for dc in range(DC):
                                nc.sync.dma_start(kT_blk[:, dc, :],
                                                  kT_r[:, dc, b * NB:(b + 1) * NB])
                        else:
                            nc.sync.dma_start(kT_blk[:],
                                              kT_r[:, :, b * NB:(b + 1) * NB])
                    v_blk = vpool.tile([P, NCX, D], mdt)
                    nc.sync.dma_start(v_blk[:], v_r[:, b * NCX:(b + 1) * NCX, :])

                    pT = ppool.tile([P, NCX, M], mdt)
                    if cfg.get("score_quad"):
                        # Four 256-col matmuls per kT weight load (m-halves
                        # paired), psum banks in AABB order: weight reuse
                        # without per-matmul bank alternation. One
                        # bank-clearing start per BANK (start zeroes the
                        # whole bank's has_written bits); the other half's
                        # first matmul overwrites-because-unwritten.
                        for ncx in range(NCX):
                            sTa = spsum.tile([P, 2, 256], f32, tag="sq",
                                             name="sTa")
                            sTb = spsum.tile([P, 2, 256], f32, tag="sq",
                                             name="sTb")
                            for dc in range(DC):
                                kw = mmcast(
                                    kT_blk[:, dc, ncx * P:(ncx + 1) * P])
                                for st, half in ((sTa, 0), (sTa, 1),
                                                 (sTb, 2), (sTb, 3)):
                                    nc.tensor.matmul(
                                        st[:, half % 2, :], kw,
                                        mmcast(qT_sb[:, dc,
                                               half * 256:(half + 1) * 256]),
                                        start=(dc == 0 and half % 2 == 0),
                                        stop=(dc == DC - 1),
                                        skip_group_check=True,
                                    )
                            for st, mh0 in ((sTa, 0), (sTb, 1)):
                                for h2 in range(2):
                                    m0 = mh0 * 512 + h2 * 256
                                    nc.scalar.activation(
                                        pT[:, ncx, m0:m0 + 256],
                                        st[:, h2, :], Exp,
                                        scale=float(SCALE),
                                    )
                    else:
                        for mh in range(NMH):
                            for ncx in range(NCX):
                                sT = spsum.tile([P, MHW], f32, tag="sT1")
                                for dc in range(DC):
                                    nc.tensor.matmul(
                                        sT[:],
                                        mmcast(kT_blk[:, dc, ncx * P:(ncx + 1) * P]),
                                        mmcast(qT_sb[:, dc, mh * MHW:(mh + 1) * MHW]),
                                        start=(dc == 0), stop=(dc == DC - 1),
                                    )
                                nc.scalar.activation(
                                    pT[:, ncx, mh * MHW:(mh + 1) * MHW], sT[:],
                                    Exp, scale=float(SCALE),
                                )

                    l_ps = lpsum.tile([P, MTS], f32)
                    for mt in range(MTS):
                        o_ps = [opsum.tile([P, OJW], f32, tag="ops",
                                           name=f"o_ps{oj}")
                                for oj in range(NOJ)]
                        if cfg["oj_major"]:
                            # Same-bank matmuls back-to-back: all ncx chunks
                            # for one output bank before switching banks
                            # (weights reload per MM; LDW hides in the
                            # reorder window).
                            for oj in range(NOJ):
                                for ncx in range(NCX):
                                    pw = mmcast(pT[:, ncx, mt * P:(mt + 1) * P])
                                    nc.tensor.matmul(
                                        o_ps[oj][:], pw,
                                        mmcast(v_blk[:, ncx,
                                               oj * OJW:(oj + 1) * OJW]),
                                        start=(ncx == 0), stop=(ncx == NCX - 1),
                                    )
                            for ncx in range(NCX):
                                pw = mmcast(pT[:, ncx, mt * P:(mt + 1) * P])
                                nc.tensor.matmul(
                                    l_ps[:, mt:mt + 1], pw, mmcast(ones[:]),
                                    start=(ncx == 0), stop=(ncx == NCX - 1),
                                    skip_group_check=True,
                                )
                        else:
                            for ncx in range(NCX):
                                pw = mmcast(pT[:, ncx, mt * P:(mt + 1) * P])
                                for oj in range(NOJ):
                                    nc.tensor.matmul(
                                        o_ps[oj][:], pw,
                                        mmcast(v_blk[:, ncx,
                                               oj * OJW:(oj + 1) * OJW]),
                                        start=(ncx == 0), stop=(ncx == NCX - 1),
                                    )
                                nc.tensor.matmul(
                                    l_ps[:, mt:mt + 1], pw, mmcast(ones[:]),
                                    start=(ncx == 0), stop=(ncx == NCX - 1),
                                    skip_group_check=True,
                                )
                        halves = [(o_ps[oj], slice(oj * OJW, (oj + 1) * OJW))
                                  for oj in range(NOJ)]
                        if b == 0:
                            for ops, js in halves:
                                nc.vector.tensor_copy(o_acc[:, mt, js], ops[:])
                        elif not last:
                            for ops, js in halves:
                                nc.vector.tensor_add(o_acc[:, mt, js],
                                                     o_acc[:, mt, js], ops[:])
                        else:
                            # fused finalization: per-m-tile l total,
                            # reciprocal, o total, scale, store.
                            l_fin = fpool.tile([P, 1], f32, tag="lfin")
                            nc.vector.tensor_add(l_fin[:], l_acc[:, mt:mt + 1],
                                                 l_ps[:, mt:mt + 1])
                            rcp = fpool.tile([P, 1], f32, tag="rcp")
                            nc.vector.reciprocal(rcp[:], l_fin[:])
                            o_out = fpool.tile([P, D], f32, tag="oout")
                            for ops, js in halves:
                                nc.vector.tensor_add(o_out[:, js],
                                                     o_acc[:, mt, js], ops[:])
                            nc.vector.tensor_scalar_mul(o_out[:], o_out[:],
                                                        rcp[:])
                            nc.sync.dma_start(o_r[:, mt, :], o_out[:])
                    if b == 0:
                        nc.vector.tensor_copy(l_acc[:], l_ps[:])
                    elif not last:
                        nc.vector.tensor_add(l_acc[:], l_acc[:], l_ps[:])

            loop_n = cfg.get("loop_n", 1)
            if loop_n > 1:
                # Hardware loop around the whole pass: used for timing
                # (T(loop=a) - T(loop=b) isolates pure device pass time from
                # the large, jittery dispatch overhead).
                with tc.For_i(0, loop_n, 1):
                    emit_pass(first_pass=False)
            else:
                for rep in range(reps):
                    emit_pass(first_pass=(rep == 0))

    # Scrub residual caller tracebacks (Tile's exit path captures one even
    # with disable_frame_to_traceback) so the BIR — and therefore the NEFF
    # compile-cache key — is identical from any call site.
    import dataclasses
    for bb in nc.m.functions[0].blocks:
        for inst in bb.instructions:
            d = inst.debug
            if d is not None and d.ant_traceback is not None:
                inst.debug = dataclasses.replace(d, ant_traceback=None)

    nc.finalize()
    return nc


def _get_exec(reps=1, cfg=None):
    """Build (once) and cache a jitted SPMD executor whose module runs
    `reps` chained attention passes. Returns (fn, in_names, out_names,
    out_avals); fn(*global_inputs, *global_zero_outs) -> global outputs."""
    cfg = dict(CFG if cfg is None else cfg)
    key = ("exec", tuple(sorted(cfg.items())), reps)
    if key in _cache:
        return _cache[key]

    import jax
    from jax.sharding import Mesh, PartitionSpec
    from jax.experimental.shard_map import shard_map
    import concourse.mybir as mybir
    from concourse import bass2jax

    nckey = ("nc",) + key[1:]
    if nckey not in _cache:
        _cache[nckey] = _build(cfg, reps)
    nc = _cache[nckey]

    bass2jax.install_neuronx_cc_hook()

    partition_name = nc.partition_id_tensor.name if nc.partition_id_tensor else None
    in_names, out_names, out_avals = [], [], []
    for alloc in nc.m.functions[0].allocations:
        if not isinstance(alloc, mybir.MemoryLocationSet):
            continue
        name = alloc.memorylocations[0].name
        if alloc.kind == "ExternalInput":
            if name != partition_name:
                in_names.append(name)
        elif alloc.kind == "ExternalOutput":
            out_names.append(name)
            out_avals.append(jax.core.ShapedArray(
                tuple(alloc.tensor_shape), mybir.dt.np(alloc.dtype)))
    n_params = len(in_names)
    n_outs = len(out_names)
    bind_names = tuple(in_names + out_names + (
        [partition_name] if partition_name else []))

    def _body(*args):
        operands = list(args)
        if partition_name is not None:
            operands.append(bass2jax.partition_id_tensor())
        outs = bass2jax._bass_exec_p.bind(
            *operands,
            out_avals=tuple(out_avals),
            in_names=bind_names,
            out_names=tuple(out_names),
            lowering_input_output_aliases=(),
            sim_require_finite=True,
            sim_require_nnan=True,
            nc=nc,
        )
        return tuple(outs)

    devices = jax.devices()[:NCORES]
    mesh = Mesh(np.asarray(devices), ("core",))
    donate = tuple(range(n_params, n_params + n_outs))
    # qT is sharded along cores; kT and v are replicated (spec None), so the
    # host passes ONE copy instead of materializing 8.
    in_spec_map = {"qT": PartitionSpec("core"), "kT": PartitionSpec(),
                   "v": PartitionSpec()}
    fn = jax.jit(shard_map(
        _body, mesh=mesh,
        in_specs=tuple(in_spec_map[nm] for nm in in_names)
        + (PartitionSpec("core"),) * n_outs,
        out_specs=(PartitionSpec("core"),) * n_outs,
        check_rep=False,
    ), donate_argnums=donate, keep_unused=True)
    _cache[key] = (fn, in_names, out_names, out_avals)
    return _cache[key]


def _prep_inputs(q, k, v):
    """Per-core host preprocessing -> dict name -> global concat array."""
    npdt = (ml_dtypes.bfloat16 if CFG["mm_dtype"] == "bf16" else np.float32)
    # Cast BEFORE transposing: the transpose-copy then moves half the bytes.
    kb = np.asarray(k).astype(npdt)
    kT = np.ascontiguousarray(kb.T)
    vv = np.ascontiguousarray(np.asarray(v).astype(npdt))
    qb = np.asarray(q).astype(npdt)
    qT_g = np.ascontiguousarray(
        qb.reshape(NCORES, M, D).transpose(0, 2, 1)).reshape(NCORES * D, M)
    # kT and v are replicated by the executor (in_spec PartitionSpec()),
    # so a single copy suffices here.
    return {"qT": qT_g, "kT": kT, "v": vv}


def _device_zeros(out_avals):
    """Per-call donated output buffers, created on device (no host transfer)."""
    import jax
    import jax.numpy as jnp
    from jax.sharding import Mesh, NamedSharding, PartitionSpec

    if "zfn" not in _cache:
        mesh = Mesh(np.asarray(jax.devices()[:NCORES]), ("core",))
        shard = NamedSharding(mesh, PartitionSpec("core"))
        shapes = [((NCORES * a.shape[0], *a.shape[1:]), a.dtype)
                  for a in out_avals]
        _cache["zfn"] = jax.jit(
            lambda: tuple(jnp.zeros(s, d) for s, d in shapes),
            out_shardings=(shard,) * len(shapes))
    return _cache["zfn"]()


def kernel(q, k, v):
    fn, in_names, out_names, out_avals = _get_exec(reps=1)
    global_ins = _prep_inputs(q, k, v)
    outs = fn(*[global_ins[nm] for nm in in_names], *_device_zeros(out_avals))
    o = np.asarray(outs[out_names.index("o")])
    return o.reshape(NCORES * M, D)



# revision 5
# speedup vs baseline: 1.0077x; 1.0077x over previous
"""Dense attention kernel for Trainium2, 8 NeuronCores (SPMD).

Problem: q,k,v [8192, 1024] fp32; out = softmax(q @ k.T / sqrt(1024)) @ v.

Strategy (sequence-parallel over q, per the sharding hint):
  - Core c owns q rows [c*1024, (c+1)*1024); k and v are replicated.
  - Host pre-transposes: each core receives qT [D, M]=[1024, 1024] (its q
    shard transposed) and kT [D, N]=[1024, 8192] (k transposed), so the
    contraction dim D is the SBUF partition dim for both matmul operands
    and no on-chip transposes are needed anywhere.
  - Scores are computed TRANSPOSED: sT[n, m] = sum_d kT[d, n] * qT[d, m]
    (lhsT = kT chunk, rhs = qT chunk). The softmax numerator
    pT = exp(sT / 32) then already has the kv dim n on partitions, which is
    exactly the lhsT layout the second matmul needs: o[m, j] += pT.T @ v.
  - No running max: scores/32 ~ N(0,1), max over 8192 ~ 4.3, so exp() is
    bounded by ~e^5 — no overflow risk in fp32, and softmax is shift
    invariant so the result matches the reference.
  - The softmax denominator l[m] = sum_n pT[n, m] falls out of a 1-column
    matmul against a ones vector, accumulated in PSUM alongside o.
  - In the last kv block, finalization (l add, reciprocal, scale, store) is
    fused per m-tile so the tail pipelines instead of serializing.

kv is streamed once per core in blocks of NB columns; o/l accumulate in
SBUF fp32 across blocks.

The executor mirrors concourse.bass2jax.run_bass_via_pjrt but caches the
jitted computation (run_bass_via_pjrt re-traces per call). `reps` unrolls
the whole attention pass inside the module for steady-state timing.
"""

import numpy as np
import ml_dtypes

# ---- problem geometry (hardcoded per contract) ----
N = 8192
D = 1024
NCORES = 8
M = N // NCORES  # 1024 q rows per core
P = 128
DC = D // P  # 8 contraction chunks

SCALE = 1.0 / np.sqrt(np.float32(D))

# Structure knobs (A/B tested on HW via bench_kernel.py):
#   nb       — kv block columns streamed per phase
#   mhw      — rhs (moving operand) width per scores matmul
#   ojw      — rhs width per output matmul
#   sps/ops  — PSUM pool buffer counts for scores / output phases
#   kt_split — per-dc DMA split of every kT block (not just b=0)
DEFAULT_CFG = dict(
    mm_dtype="bf16",
    nb=512,
    mhw=512,
    ojw=512,
    sps_bufs=2,
    # 5 output-phase PSUM banks (2+5+1=8 total): measured ~570 vs ~600
    # us/pass at 4 across three interleaved loop-slope A/B sessions; deeper
    # rotation lets one more m-tile's matmuls start before a prior tile's
    # psum drain completes. 3 is catastrophically slower (breaks the
    # same-weight matmul trio placement).
    ops_bufs=5,
    # Quad-buffered pT: lets block b+1's exp outputs land while block
    # b-1's output-phase matmuls still read their pT, with one more block
    # in flight than the earlier triple-buffer setting (3 measured ~584 vs
    # ~605 us/pass at 2; 4 measured q25 528.8 vs 546.5 us vs 3 in a
    # 22-round interleaved A/B — the largest consistent delta of any knob).
    pt_bufs=4,
    kt_split=False,
    oj_major=False,
    # 4-deep finalization-tile rotation: lets more per-m-tile l/rcp/scale
    # chains overlap the last block's out-phase matmuls. Nominally best in
    # two independent 10/22-round interleaved A/B sessions (q25 555.8/559.1
    # vs control 557.2/574.8 us); never worse than control beyond noise.
    fin_bufs=4,
)

CFG = dict(DEFAULT_CFG)

_cache = {}


def _build(cfg, reps=1):
    import concourse.bass as bass
    import concourse.tile as tile
    import concourse.mybir as mybir
    from concourse import bacc

    f32 = mybir.dt.float32
    if cfg["mm_dtype"] == "bf16":
        mdt = mybir.dt.bfloat16
        mmcast = lambda ap: ap
    else:
        mdt = mybir.dt.float32
        mmcast = lambda ap: ap.bitcast(mybir.dt.float32r)

    NB = cfg["nb"]
    NBLK = N // NB
    NCX = NB // P          # kv partition-chunks per block
    MTS = M // P           # m-tiles per core
    MHW = cfg["mhw"]       # scores moving-operand width
    NMH = M // MHW
    OJW = cfg["ojw"]       # out-phase moving-operand width
    NOJ = D // OJW

    # disable_frame_to_traceback keeps caller tracebacks out of the BIR so
    # the build (and the NEFF-cache key) is identical from any call site.
    nc = bacc.Bacc("TRN2", target_bir_lowering=False, debug=False,
                   num_devices=NCORES, disable_frame_to_traceback=True)
    qT_d = nc.declare_dram_parameter("qT", [D, M], mdt, isOutput=False)
    kT_d = nc.declare_dram_parameter("kT", [D, N], mdt, isOutput=False)
    v_d = nc.declare_dram_parameter("v", [N, D], mdt, isOutput=False)
    o_d = nc.declare_dram_parameter("o", [M, D], f32, isOutput=True)

    qT_r = qT_d.rearrange("(dc p) m -> p dc m", p=P)
    kT_r = kT_d.rearrange("(dc p) n -> p dc n", p=P)
    v_r = v_d.rearrange("(nb p) j -> p nb j", p=P)
    o_r = o_d.rearrange("(mt p) j -> p mt j", p=P)

    Exp = mybir.ActivationFunctionType.Exp

    wide = cfg["mm_dtype"] != "bf16"
    qabufs = cfg.get("qa_bufs") or (1 if (reps == 1 or wide) else 2)
    finbufs = cfg.get("fin_bufs", 2)
    kvbufs = cfg.get("kv_bufs") or (2 if (wide or NB >= 1024) else 3)

    with tile.TileContext(nc) as tc:
        with (
            tc.tile_pool(name="const", bufs=1) as cpool,
            tc.tile_pool(name="qT", bufs=qabufs) as qpool,
            tc.tile_pool(name="acc", bufs=qabufs) as apool,
            tc.tile_pool(name="kT", bufs=kvbufs) as kpool,
            tc.tile_pool(name="v", bufs=kvbufs) as vpool,
            tc.tile_pool(name="pT", bufs=cfg.get("pt_bufs", 2)) as ppool,
            tc.tile_pool(name="fin", bufs=finbufs) as fpool,
            tc.tile_pool(name="sps", bufs=cfg["sps_bufs"], space="PSUM") as spsum,
            tc.tile_pool(name="ops", bufs=cfg["ops_bufs"], space="PSUM") as opsum,
            tc.tile_pool(name="lps", bufs=1, space="PSUM") as lpsum,
        ):
            ones = cpool.tile([P, 1], mdt)
            nc.vector.memset(ones[:], 1.0)

            def emit_pass(first_pass):
                # Per-dc DMA split so the first matmuls start as soon as the
                # first contraction chunks land, not after the whole 2 MB.
                qT_sb = qpool.tile([P, DC, M], mdt, name="qT_sb")
                kT_b0 = None
                if first_pass:
                    # First pass: interleave qT and kT(b=0) chunk DMAs on the
                    # (FIFO) HWDGE ring so the first score chain's dc=0
                    # operands land ~1 us in, not after the whole 2 MB of qT.
                    kT_b0 = kpool.tile([P, DC, NB], mdt, name="kT_blk")
                    for dc in range(DC):
                        nc.sync.dma_start(qT_sb[:, dc, :], qT_r[:, dc, :])
                        nc.sync.dma_start(kT_b0[:, dc, :],
                                          kT_r[:, dc, 0:NB])
                else:
                    for dc in range(DC):
                        nc.sync.dma_start(qT_sb[:, dc, :], qT_r[:, dc, :])

                o_acc = apool.tile([P, MTS, D], f32, name="o_acc")
                l_acc = apool.tile([P, MTS], f32)

                for b in range(NBLK):
                    last = b == NBLK - 1
                    if b == 0 and kT_b0 is not None:
                        kT_blk = kT_b0
                    else:
                        kT_blk = kpool.tile([P, DC, NB], mdt)
                        if cfg["kt_split"]:
                            for dc in range(DC):
                                nc.sync.dma_start(kT_blk[:, dc, :],
                                                  kT_r[:, dc, b * NB:(b + 1) * NB])
                        else:
                            nc.sync.dma_start(kT_blk[:],
                                              kT_r[:, :, b * NB:(b + 1) * NB])
                    v_blk = vpool.tile([P, NCX, D], mdt)
                    nc.sync.dma_start(v_blk[:], v_r[:, b * NCX:(b + 1) * NCX, :])

                    pT = ppool.tile([P, NCX, M], mdt)
                    if cfg.get("score_quad"):
                        # Four 256-col matmuls per kT weight load (m-halves
                        # paired), psum banks in AABB order: weight reuse
                        # without per-matmul bank alternation. One
                        # bank-clearing start per BANK (start zeroes the
                        # whole bank's has_written bits); the other half's
                        # first matmul overwrites-because-unwritten.
                        for ncx in range(NCX):
                            sTa = spsum.tile([P, 2, 256], f32, tag="sq",
                                             name="sTa")
                            sTb = spsum.tile([P, 2, 256], f32, tag="sq",
                                             name="sTb")
                            for dc in range(DC):
                                kw = mmcast(
                                    kT_blk[:, dc, ncx * P:(ncx + 1) * P])
                                for st, half in ((sTa, 0), (sTa, 1),
                                                 (sTb, 2), (sTb, 3)):
                                    nc.tensor.matmul(
                                        st[:, half % 2, :], kw,
                                        mmcast(qT_sb[:, dc,
                                               half * 256:(half + 1) * 256]),
                                        start=(dc == 0 and half % 2 == 0),
                                        stop=(dc == DC - 1),
                                        skip_group_check=True,
                                    )
                            for st, mh0 in ((sTa, 0), (sTb, 1)):
                                for h2 in range(2):
                                    m0 = mh0 * 512 + h2 * 256
                                    nc.scalar.activation(
                                        pT[:, ncx, m0:m0 + 256],
                                        st[:, h2, :], Exp,
                                        scale=float(SCALE),
                                    )
                    else:
                        for mh in range(NMH):
                            for ncx in range(NCX):
                                sT = spsum.tile([P, MHW], f32, tag="sT1")
                                for dc in range(DC):
                                    nc.tensor.matmul(
                                        sT[:],
                                        mmcast(kT_blk[:, dc, ncx * P:(ncx + 1) * P]),
                                        mmcast(qT_sb[:, dc, mh * MHW:(mh + 1) * MHW]),
                                        start=(dc == 0), stop=(dc == DC - 1),
                                    )
                                nc.scalar.activation(
                                    pT[:, ncx, mh * MHW:(mh + 1) * MHW], sT[:],
                                    Exp, scale=float(SCALE),
                                )

                    l_ps = lpsum.tile([P, MTS], f32)
                    for mt in range(MTS):
                        o_ps = [opsum.tile([P, OJW], f32, tag="ops",
                                           name=f"o_ps{oj}")
                                for oj in range(NOJ)]
                        if cfg["oj_major"]:
                            # Same-bank matmuls back-to-back: all ncx chunks
                            # for one output bank before switching banks
                            # (weights reload per MM; LDW hides in the
                            # reorder window).
                            for oj in range(NOJ):
                                for ncx in range(NCX):
                                    pw = mmcast(pT[:, ncx, mt * P:(mt + 1) * P])
                                    nc.tensor.matmul(
                                        o_ps[oj][:], pw,
                                        mmcast(v_blk[:, ncx,
                                               oj * OJW:(oj + 1) * OJW]),
                                        start=(ncx == 0), stop=(ncx == NCX - 1),
                                    )
                            for ncx in range(NCX):
                                pw = mmcast(pT[:, ncx, mt * P:(mt + 1) * P])
                                nc.tensor.matmul(
                                    l_ps[:, mt:mt + 1], pw, mmcast(ones[:]),
                                    start=(ncx == 0), stop=(ncx == NCX - 1),
                                    skip_group_check=True,
                                )
                        else:
                            for ncx in range(NCX):
                                pw = mmcast(pT[:, ncx, mt * P:(mt + 1) * P])
                                for oj in range(NOJ):
                                    nc.tensor.matmul(
                                        o_ps[oj][:], pw,
                                        mmcast(v_blk[:, ncx,
                                               oj * OJW:(oj + 1) * OJW]),
                                        start=(ncx == 0), stop=(ncx == NCX - 1),
                                    )
                                nc.tensor.matmul(
                                    l_ps[:, mt:mt + 1], pw, mmcast(ones[:]),
                                    start=(ncx == 0), stop=(ncx == NCX - 1),
                                    skip_group_check=True,
                                )
                        halves = [(o_ps[oj], slice(oj * OJW, (oj + 1) * OJW))
                                  for oj in range(NOJ)]
                        if b == 0:
                            for ops, js in halves:
                                nc.vector.tensor_copy(o_acc[:, mt, js], ops[:])
                        elif not last:
                            for ops, js in halves:
                                nc.vector.tensor_add(o_acc[:, mt, js],
                                                     o_acc[:, mt, js], ops[:])
                        else:
                            # fused finalization: per-m-tile l total,
                            # reciprocal, o total, scale, store.
                            l_fin = fpool.tile([P, 1], f32, tag="lfin")
                            nc.vector.tensor_add(l_fin[:], l_acc[:, mt:mt + 1],
                                                 l_ps[:, mt:mt + 1])
                            rcp = fpool.tile([P, 1], f32, tag="rcp")
                            nc.vector.reciprocal(rcp[:], l_fin[:])
                            o_out = fpool.tile([P, D], f32, tag="oout")
                            for ops, js in halves:
                                nc.vector.tensor_add(o_out[:, js],
                                                     o_acc[:, mt, js], ops[:])
                            nc.vector.tensor_scalar_mul(o_out[:], o_out[:],
                                                        rcp[:])
                            nc.sync.dma_start(o_r[:, mt, :], o_out[:])
                    if b == 0:
                        nc.vector.tensor_copy(l_acc[:], l_ps[:])
                    elif not last:
                        nc.vector.tensor_add(l_acc[:], l_acc[:], l_ps[:])

            loop_n = cfg.get("loop_n", 1)
            if loop_n > 1:
                # Hardware loop around the whole pass: used for timing
                # (T(loop=a) - T(loop=b) isolates pure device pass time from
                # the large, jittery dispatch overhead).
                with tc.For_i(0, loop_n, 1):
                    emit_pass(first_pass=False)
            else:
                for rep in range(reps):
                    emit_pass(first_pass=(rep == 0))

    # Scrub residual caller tracebacks (Tile's exit path captures one even
    # with disable_frame_to_traceback) so the BIR — and therefore the NEFF
    # compile-cache key — is identical from any call site.
    import dataclasses
    for bb in nc.m.functions[0].blocks:
        for inst in bb.instructions:
            d = inst.debug
            if d is not None and d.ant_traceback is not None:
                inst.debug = dataclasses.replace(d, ant_traceback=None)

    nc.finalize()
    return nc


def _get_exec(reps=1, cfg=None):
    """Build (once) and cache a jitted SPMD executor whose module runs
    `reps` chained attention passes. Returns (fn, in_names, out_names,
    out_avals); fn(*global_inputs, *global_zero_outs) -> global outputs."""
    cfg = dict(CFG if cfg is None else cfg)
    key = ("exec", tuple(sorted(cfg.items())), reps)
    if key in _cache:
        return _cache[key]

    import jax
    from jax.sharding import Mesh, PartitionSpec
    from jax.experimental.shard_map import shard_map
    import concourse.mybir as mybir
    from concourse import bass2jax

    nckey = ("nc",) + key[1:]
    if nckey not in _cache:
        _cache[nckey] = _build(cfg, reps)
    nc = _cache[nckey]

    bass2jax.install_neuronx_cc_hook()

    partition_name = nc.partition_id_tensor.name if nc.partition_id_tensor else None
    in_names, out_names, out_avals = [], [], []
    for alloc in nc.m.functions[0].allocations:
        if not isinstance(alloc, mybir.MemoryLocationSet):
            continue
        name = alloc.memorylocations[0].name
        if alloc.kind == "ExternalInput":
            if name != partition_name:
                in_names.append(name)
        elif alloc.kind == "ExternalOutput":
            out_names.append(name)
            out_avals.append(jax.core.ShapedArray(
                tuple(alloc.tensor_shape), mybir.dt.np(alloc.dtype)))
    n_params = len(in_names)
    n_outs = len(out_names)
    bind_names = tuple(in_names + out_names + (
        [partition_name] if partition_name else []))

    def _body(*args):
        operands = list(args)
        if partition_name is not None:
            operands.append(bass2jax.partition_id_tensor())
        outs = bass2jax._bass_exec_p.bind(
            *operands,
            out_avals=tuple(out_avals),
            in_names=bind_names,
            out_names=tuple(out_names),
            lowering_input_output_aliases=(),
            sim_require_finite=True,
            sim_require_nnan=True,
            nc=nc,
        )
        return tuple(outs)

    devices = jax.devices()[:NCORES]
    mesh = Mesh(np.asarray(devices), ("core",))
    donate = tuple(range(n_params, n_params + n_outs))
    # qT is sharded along cores; kT and v are replicated (spec None), so the
    # host passes ONE copy instead of materializing 8.
    in_spec_map = {"qT": PartitionSpec("core"), "kT": PartitionSpec(),
                   "v": PartitionSpec()}
    fn = jax.jit(shard_map(
        _body, mesh=mesh,
        in_specs=tuple(in_spec_map[nm] for nm in in_names)
        + (PartitionSpec("core"),) * n_outs,
        out_specs=(PartitionSpec("core"),) * n_outs,
        check_rep=False,
    ), donate_argnums=donate, keep_unused=True)
    _cache[key] = (fn, in_names, out_names, out_avals)
    return _cache[key]


def _prep_inputs(q, k, v):
    """Per-core host preprocessing -> dict name -> global concat array."""
    npdt = (ml_dtypes.bfloat16 if CFG["mm_dtype"] == "bf16" else np.float32)
    # Cast BEFORE transposing: the transpose-copy then moves half the bytes.
    kb = np.asarray(k).astype(npdt)
    kT = np.ascontiguousarray(kb.T)
    vv = np.ascontiguousarray(np.asarray(v).astype(npdt))
    qb = np.asarray(q).astype(npdt)
    qT_g = np.ascontiguousarray(
        qb.reshape(NCORES, M, D).transpose(0, 2, 1)).reshape(NCORES * D, M)
    # kT and v are replicated by the executor (in_spec PartitionSpec()),
    # so a single copy suffices here.
    return {"qT": qT_g, "kT": kT, "v": vv}


def _device_zeros(out_avals):
    """Per-call donated output buffers, created on device (no host transfer)."""
    import jax
    import jax.numpy as jnp
    from jax.sharding import Mesh, NamedSharding, PartitionSpec

    if "zfn" not in _cache:
        mesh = Mesh(np.asarray(jax.devices()[:NCORES]), ("core",))
        shard = NamedSharding(mesh, PartitionSpec("core"))
        shapes = [((NCORES * a.shape[0], *a.shape[1:]), a.dtype)
                  for a in out_avals]
        _cache["zfn"] = jax.jit(
            lambda: tuple(jnp.zeros(s, d) for s, d in shapes),
            out_shardings=(shard,) * len(shapes))
    return _cache["zfn"]()


def kernel(q, k, v):
    fn, in_names, out_names, out_avals = _get_exec(reps=1)
    global_ins = _prep_inputs(q, k, v)
    outs = fn(*[global_ins[nm] for nm in in_names], *_device_zeros(out_avals))
    o = np.asarray(outs[out_names.index("o")])
    return o.reshape(NCORES * M, D)



# revision 6
# speedup vs baseline: 1.0336x; 1.0258x over previous
"""Dense attention kernel for Trainium2, 8 NeuronCores (SPMD).

Problem: q,k,v [8192, 1024] fp32; out = softmax(q @ k.T / sqrt(1024)) @ v.

Strategy (sequence-parallel over q, per the sharding hint):
  - Core c owns q rows [c*1024, (c+1)*1024); k and v are replicated.
  - Host pre-transposes: each core receives qT [D, M]=[1024, 1024] (its q
    shard transposed) and kT [D, N]=[1024, 8192] (k transposed), so the
    contraction dim D is the SBUF partition dim for both matmul operands
    and no on-chip transposes are needed anywhere.
  - Scores are computed TRANSPOSED: sT[n, m] = sum_d kT[d, n] * qT[d, m]
    (lhsT = kT chunk, rhs = qT chunk). The softmax numerator
    pT = exp(sT / 32) then already has the kv dim n on partitions, which is
    exactly the lhsT layout the second matmul needs: o[m, j] += pT.T @ v.
  - No running max: scores/32 ~ N(0,1), max over 8192 ~ 4.3, so exp() is
    bounded by ~e^5 — no overflow risk in fp32, and softmax is shift
    invariant so the result matches the reference.
  - The softmax denominator l[m] = sum_n pT[n, m] falls out of a 1-column
    matmul against a ones vector, accumulated in PSUM alongside o.
  - In the last kv block, finalization (l add, reciprocal, scale, store) is
    fused per m-tile so the tail pipelines instead of serializing.

kv is streamed once per core in blocks of NB columns; o/l accumulate in
SBUF fp32 across blocks.

The executor mirrors concourse.bass2jax.run_bass_via_pjrt but caches the
jitted computation (run_bass_via_pjrt re-traces per call). `reps` unrolls
the whole attention pass inside the module for steady-state timing.
"""

import numpy as np
import ml_dtypes

# ---- problem geometry (hardcoded per contract) ----
N = 8192
D = 1024
NCORES = 8
M = N // NCORES  # 1024 q rows per core
P = 128
DC = D // P  # 8 contraction chunks

SCALE = 1.0 / np.sqrt(np.float32(D))

# Structure knobs (A/B tested on HW via bench_kernel.py):
#   nb       — kv block columns streamed per phase
#   mhw      — rhs (moving operand) width per scores matmul
#   ojw      — rhs width per output matmul
#   sps/ops  — PSUM pool buffer counts for scores / output phases
#   kt_split — per-dc DMA split of every kT block (not just b=0)
DEFAULT_CFG = dict(
    mm_dtype="bf16",
    nb=512,
    mhw=512,
    ojw=512,
    sps_bufs=2,
    # 5 output-phase PSUM banks (2+5+1=8 total): measured ~570 vs ~600
    # us/pass at 4 across three interleaved loop-slope A/B sessions; deeper
    # rotation lets one more m-tile's matmuls start before a prior tile's
    # psum drain completes. 3 is catastrophically slower (breaks the
    # same-weight matmul trio placement).
    ops_bufs=5,
    # Quad-buffered pT: lets block b+1's exp outputs land while block
    # b-1's output-phase matmuls still read their pT, with one more block
    # in flight than the earlier triple-buffer setting (3 measured ~584 vs
    # ~605 us/pass at 2; 4 measured q25 528.8 vs 546.5 us vs 3 in a
    # 22-round interleaved A/B — the largest consistent delta of any knob).
    pt_bufs=4,
    kt_split=False,
    oj_major=False,
    # 4-deep finalization-tile rotation: lets more per-m-tile l/rcp/scale
    # chains overlap the last block's out-phase matmuls. Nominally best in
    # two independent 10/22-round interleaved A/B sessions (q25 555.8/559.1
    # vs control 557.2/574.8 us); never worse than control beyond noise.
    fin_bufs=4,
    # Double-buffered qT/o_acc pools: decouples loop iterations (pass i+1's
    # qT DMA + first scores no longer wait on pass i's finalization tail).
    # CoreSim models it as neutral in steady state; on HW it was q25-better
    # in 2 of 3 interleaved A/B windows (549.3 vs 579.2; 564.0 vs 574.8;
    # 562.4 vs 557.2 us), avg ~-12 us.
    qa_bufs=2,
)

CFG = dict(DEFAULT_CFG)

_cache = {}


def _build(cfg, reps=1):
    import concourse.bass as bass
    import concourse.tile as tile
    import concourse.mybir as mybir
    from concourse import bacc

    f32 = mybir.dt.float32
    if cfg["mm_dtype"] == "bf16":
        mdt = mybir.dt.bfloat16
        mmcast = lambda ap: ap
    else:
        mdt = mybir.dt.float32
        mmcast = lambda ap: ap.bitcast(mybir.dt.float32r)

    NB = cfg["nb"]
    NBLK = N // NB
    NCX = NB // P          # kv partition-chunks per block
    MTS = M // P           # m-tiles per core
    MHW = cfg["mhw"]       # scores moving-operand width
    NMH = M // MHW
    OJW = cfg["ojw"]       # out-phase moving-operand width
    NOJ = D // OJW

    # disable_frame_to_traceback keeps caller tracebacks out of the BIR so
    # the build (and the NEFF-cache key) is identical from any call site.
    nc = bacc.Bacc("TRN2", target_bir_lowering=False, debug=False,
                   num_devices=NCORES, disable_frame_to_traceback=True)
    qT_d = nc.declare_dram_parameter("qT", [D, M], mdt, isOutput=False)
    kT_d = nc.declare_dram_parameter("kT", [D, N], mdt, isOutput=False)
    v_d = nc.declare_dram_parameter("v", [N, D], mdt, isOutput=False)
    o_d = nc.declare_dram_parameter("o", [M, D], f32, isOutput=True)

    qT_r = qT_d.rearrange("(dc p) m -> p dc m", p=P)
    kT_r = kT_d.rearrange("(dc p) n -> p dc n", p=P)
    v_r = v_d.rearrange("(nb p) j -> p nb j", p=P)
    o_r = o_d.rearrange("(mt p) j -> p mt j", p=P)

    Exp = mybir.ActivationFunctionType.Exp

    wide = cfg["mm_dtype"] != "bf16"
    qabufs = cfg.get("qa_bufs") or (1 if (reps == 1 or wide) else 2)
    finbufs = cfg.get("fin_bufs", 2)
    kvbufs = cfg.get("kv_bufs") or (2 if (wide or NB >= 1024) else 3)

    with tile.TileContext(nc) as tc:
        with (
            tc.tile_pool(name="const", bufs=1) as cpool,
            tc.tile_pool(name="qT", bufs=qabufs) as qpool,
            tc.tile_pool(name="acc", bufs=qabufs) as apool,
            tc.tile_pool(name="kT", bufs=kvbufs) as kpool,
            tc.tile_pool(name="v", bufs=kvbufs) as vpool,
            tc.tile_pool(name="pT", bufs=cfg.get("pt_bufs", 2)) as ppool,
            tc.tile_pool(name="fin", bufs=finbufs) as fpool,
            tc.tile_pool(name="sps", bufs=cfg["sps_bufs"], space="PSUM") as spsum,
            tc.tile_pool(name="ops", bufs=cfg["ops_bufs"], space="PSUM") as opsum,
            tc.tile_pool(name="lps", bufs=1, space="PSUM") as lpsum,
        ):
            ones = cpool.tile([P, 1], mdt)
            nc.vector.memset(ones[:], 1.0)

            def emit_pass(first_pass):
                # Per-dc DMA split so the first matmuls start as soon as the
                # first contraction chunks land, not after the whole 2 MB.
                qT_sb = qpool.tile([P, DC, M], mdt, name="qT_sb")
                kT_b0 = None
                if first_pass:
                    # First pass: interleave qT and kT(b=0) chunk DMAs on the
                    # (FIFO) HWDGE ring so the first score chain's dc=0
                    # operands land ~1 us in, not after the whole 2 MB of qT.
                    kT_b0 = kpool.tile([P, DC, NB], mdt, name="kT_blk")
                    for dc in range(DC):
                        nc.sync.dma_start(qT_sb[:, dc, :], qT_r[:, dc, :])
                        nc.sync.dma_start(kT_b0[:, dc, :],
                                          kT_r[:, dc, 0:NB])
                else:
                    for dc in range(DC):
                        nc.sync.dma_start(qT_sb[:, dc, :], qT_r[:, dc, :])

                o_acc = apool.tile([P, MTS, D], f32, name="o_acc")
                l_acc = apool.tile([P, MTS], f32)

                for b in range(NBLK):
                    last = b == NBLK - 1
                    if b == 0 and kT_b0 is not None:
                        kT_blk = kT_b0
                    else:
                        kT_blk = kpool.tile([P, DC, NB], mdt)
                        if cfg["kt_split"]:
                            for dc in range(DC):
                                nc.sync.dma_start(kT_blk[:, dc, :],
                                                  kT_r[:, dc, b * NB:(b + 1) * NB])
                        else:
                            nc.sync.dma_start(kT_blk[:],
                                              kT_r[:, :, b * NB:(b + 1) * NB])
                    v_blk = vpool.tile([P, NCX, D], mdt)
                    nc.sync.dma_start(v_blk[:], v_r[:, b * NCX:(b + 1) * NCX, :])

                    pT = ppool.tile([P, NCX, M], mdt)
                    if cfg.get("score_quad"):
                        # Four 256-col matmuls per kT weight load (m-halves
                        # paired), psum banks in AABB order: weight reuse
                        # without per-matmul bank alternation. One
                        # bank-clearing start per BANK (start zeroes the
                        # whole bank's has_written bits); the other half's
                        # first matmul overwrites-because-unwritten.
                        for ncx in range(NCX):
                            sTa = spsum.tile([P, 2, 256], f32, tag="sq",
                                             name="sTa")
                            sTb = spsum.tile([P, 2, 256], f32, tag="sq",
                                             name="sTb")
                            for dc in range(DC):
                                kw = mmcast(
                                    kT_blk[:, dc, ncx * P:(ncx + 1) * P])
                                for st, half in ((sTa, 0), (sTa, 1),
                                                 (sTb, 2), (sTb, 3)):
                                    nc.tensor.matmul(
                                        st[:, half % 2, :], kw,
                                        mmcast(qT_sb[:, dc,
                                               half * 256:(half + 1) * 256]),
                                        start=(dc == 0 and half % 2 == 0),
                                        stop=(dc == DC - 1),
                                        skip_group_check=True,
                                    )
                            for st, mh0 in ((sTa, 0), (sTb, 1)):
                                for h2 in range(2):
                                    m0 = mh0 * 512 + h2 * 256
                                    nc.scalar.activation(
                                        pT[:, ncx, m0:m0 + 256],
                                        st[:, h2, :], Exp,
                                        scale=float(SCALE),
                                    )
                    else:
                        for mh in range(NMH):
                            for ncx in range(NCX):
                                sT = spsum.tile([P, MHW], f32, tag="sT1")
                                for dc in range(DC):
                                    nc.tensor.matmul(
                                        sT[:],
                                        mmcast(kT_blk[:, dc, ncx * P:(ncx + 1) * P]),
                                        mmcast(qT_sb[:, dc, mh * MHW:(mh + 1) * MHW]),
                                        start=(dc == 0), stop=(dc == DC - 1),
                                    )
                                nc.scalar.activation(
                                    pT[:, ncx, mh * MHW:(mh + 1) * MHW], sT[:],
                                    Exp, scale=float(SCALE),
                                )

                    l_ps = lpsum.tile([P, MTS], f32)
                    for mt in range(MTS):
                        o_ps = [opsum.tile([P, OJW], f32, tag="ops",
                                           name=f"o_ps{oj}")
                                for oj in range(NOJ)]
                        if cfg["oj_major"]:
                            # Same-bank matmuls back-to-back: all ncx chunks
                            # for one output bank before switching banks
                            # (weights reload per MM; LDW hides in the
                            # reorder window).
                            for oj in range(NOJ):
                                for ncx in range(NCX):
                                    pw = mmcast(pT[:, ncx, mt * P:(mt + 1) * P])
                                    nc.tensor.matmul(
                                        o_ps[oj][:], pw,
                                        mmcast(v_blk[:, ncx,
                                               oj * OJW:(oj + 1) * OJW]),
                                        start=(ncx == 0), stop=(ncx == NCX - 1),
                                    )
                            for ncx in range(NCX):
                                pw = mmcast(pT[:, ncx, mt * P:(mt + 1) * P])
                                nc.tensor.matmul(
                                    l_ps[:, mt:mt + 1], pw, mmcast(ones[:]),
                                    start=(ncx == 0), stop=(ncx == NCX - 1),
                                    skip_group_check=True,
                                )
                        else:
                            for ncx in range(NCX):
                                pw = mmcast(pT[:, ncx, mt * P:(mt + 1) * P])
                                for oj in range(NOJ):
                                    nc.tensor.matmul(
                                        o_ps[oj][:], pw,
                                        mmcast(v_blk[:, ncx,
                                               oj * OJW:(oj + 1) * OJW]),
                                        start=(ncx == 0), stop=(ncx == NCX - 1),
                                    )
                                nc.tensor.matmul(
                                    l_ps[:, mt:mt + 1], pw, mmcast(ones[:]),
                                    start=(ncx == 0), stop=(ncx == NCX - 1),
                                    skip_group_check=True,
                                )
                        halves = [(o_ps[oj], slice(oj * OJW, (oj + 1) * OJW))
                                  for oj in range(NOJ)]
                        if b == 0:
                            for ops, js in halves:
                                nc.vector.tensor_copy(o_acc[:, mt, js], ops[:])
                        elif not last:
                            for ops, js in halves:
                                nc.vector.tensor_add(o_acc[:, mt, js],
                                                     o_acc[:, mt, js], ops[:])
                        else:
                            # fused finalization: per-m-tile l total,
                            # reciprocal, o total, scale, store.
                            l_fin = fpool.tile([P, 1], f32, tag="lfin")
                            nc.vector.tensor_add(l_fin[:], l_acc[:, mt:mt + 1],
                                                 l_ps[:, mt:mt + 1])
                            rcp = fpool.tile([P, 1], f32, tag="rcp")
                            nc.vector.reciprocal(rcp[:], l_fin[:])
                            o_out = fpool.tile([P, D], f32, tag="oout")
                            for ops, js in halves:
                                nc.vector.tensor_add(o_out[:, js],
                                                     o_acc[:, mt, js], ops[:])
                            nc.vector.tensor_scalar_mul(o_out[:], o_out[:],
                                                        rcp[:])
                            nc.sync.dma_start(o_r[:, mt, :], o_out[:])
                    if b == 0:
                        nc.vector.tensor_copy(l_acc[:], l_ps[:])
                    elif not last:
                        nc.vector.tensor_add(l_acc[:], l_acc[:], l_ps[:])

            loop_n = cfg.get("loop_n", 1)
            if loop_n > 1:
                # Hardware loop around the whole pass: used for timing
                # (T(loop=a) - T(loop=b) isolates pure device pass time from
                # the large, jittery dispatch overhead).
                with tc.For_i(0, loop_n, 1):
                    emit_pass(first_pass=False)
            else:
                for rep in range(reps):
                    emit_pass(first_pass=(rep == 0))

    # Scrub residual caller tracebacks (Tile's exit path captures one even
    # with disable_frame_to_traceback) so the BIR — and therefore the NEFF
    # compile-cache key — is identical from any call site.
    import dataclasses
    for bb in nc.m.functions[0].blocks:
        for inst in bb.instructions:
            d = inst.debug
            if d is not None and d.ant_traceback is not None:
                inst.debug = dataclasses.replace(d, ant_traceback=None)

    nc.finalize()
    return nc


def _get_exec(reps=1, cfg=None):
    """Build (once) and cache a jitted SPMD executor whose module runs
    `reps` chained attention passes. Returns (fn, in_names, out_names,
    out_avals); fn(*global_inputs, *global_zero_outs) -> global outputs."""
    cfg = dict(CFG if cfg is None else cfg)
    key = ("exec", tuple(sorted(cfg.items())), reps)
    if key in _cache:
        return _cache[key]

    import jax
    from jax.sharding import Mesh, PartitionSpec
    from jax.experimental.shard_map import shard_map
    import concourse.mybir as mybir
    from concourse import bass2jax

    nckey = ("nc",) + key[1:]
    if nckey not in _cache:
        _cache[nckey] = _build(cfg, reps)
    nc = _cache[nckey]

    bass2jax.install_neuronx_cc_hook()

    partition_name = nc.partition_id_tensor.name if nc.partition_id_tensor else None
    in_names, out_names, out_avals = [], [], []
    for alloc in nc.m.functions[0].allocations:
        if not isinstance(alloc, mybir.MemoryLocationSet):
            continue
        name = alloc.memorylocations[0].name
        if alloc.kind == "ExternalInput":
            if name != partition_name:
                in_names.append(name)
        elif alloc.kind == "ExternalOutput":
            out_names.append(name)
            out_avals.append(jax.core.ShapedArray(
                tuple(alloc.tensor_shape), mybir.dt.np(alloc.dtype)))
    n_params = len(in_names)
    n_outs = len(out_names)
    bind_names = tuple(in_names + out_names + (
        [partition_name] if partition_name else []))

    def _body(*args):
        operands = list(args)
        if partition_name is not None:
            operands.append(bass2jax.partition_id_tensor())
        outs = bass2jax._bass_exec_p.bind(
            *operands,
            out_avals=tuple(out_avals),
            in_names=bind_names,
            out_names=tuple(out_names),
            lowering_input_output_aliases=(),
            sim_require_finite=True,
            sim_require_nnan=True,
            nc=nc,
        )
        return tuple(outs)

    devices = jax.devices()[:NCORES]
    mesh = Mesh(np.asarray(devices), ("core",))
    donate = tuple(range(n_params, n_params + n_outs))
    # qT is sharded along cores; kT and v are replicated (spec None), so the
    # host passes ONE copy instead of materializing 8.
    in_spec_map = {"qT": PartitionSpec("core"), "kT": PartitionSpec(),
                   "v": PartitionSpec()}
    fn = jax.jit(shard_map(
        _body, mesh=mesh,
        in_specs=tuple(in_spec_map[nm] for nm in in_names)
        + (PartitionSpec("core"),) * n_outs,
        out_specs=(PartitionSpec("core"),) * n_outs,
        check_rep=False,
    ), donate_argnums=donate, keep_unused=True)
    _cache[key] = (fn, in_names, out_names, out_avals)
    return _cache[key]


def _prep_inputs(q, k, v):
    """Per-core host preprocessing -> dict name -> global concat array."""
    npdt = (ml_dtypes.bfloat16 if CFG["mm_dtype"] == "bf16" else np.float32)
    # Cast BEFORE transposing: the transpose-copy then moves half the bytes.
    kb = np.asarray(k).astype(npdt)
    kT = np.ascontiguousarray(kb.T)
    vv = np.ascontiguousarray(np.asarray(v).astype(npdt))
    qb = np.asarray(q).astype(npdt)
    qT_g = np.ascontiguousarray(
        qb.reshape(NCORES, M, D).transpose(0, 2, 1)).reshape(NCORES * D, M)
    # kT and v are replicated by the executor (in_spec PartitionSpec()),
    # so a single copy suffices here.
    return {"qT": qT_g, "kT": kT, "v": vv}


def _device_zeros(out_avals):
    """Per-call donated output buffers, created on device (no host transfer)."""
    import jax
    import jax.numpy as jnp
    from jax.sharding import Mesh, NamedSharding, PartitionSpec

    if "zfn" not in _cache:
        mesh = Mesh(np.asarray(jax.devices()[:NCORES]), ("core",))
        shard = NamedSharding(mesh, PartitionSpec("core"))
        shapes = [((NCORES * a.shape[0], *a.shape[1:]), a.dtype)
                  for a in out_avals]
        _cache["zfn"] = jax.jit(
            lambda: tuple(jnp.zeros(s, d) for s, d in shapes),
            out_shardings=(shard,) * len(shapes))
    return _cache["zfn"]()


def kernel(q, k, v):
    fn, in_names, out_names, out_avals = _get_exec(reps=1)
    global_ins = _prep_inputs(q, k, v)
    outs = fn(*[global_ins[nm] for nm in in_names], *_device_zeros(out_avals))
    o = np.asarray(outs[out_names.index("o")])
    return o.reshape(NCORES * M, D)

